# revision 26
# baseline (speedup 1.0000x reference)
"""Trainium2 Bass kernel: windowed-LSTM local attention + linear head (LBNER).

Strategy
--------
Data-parallel over batch: B=8 sequences -> 8 NeuronCores, one sequence each.
Per core everything is laid out feature-on-partitions, L=512 on the free dim:

  xT            [768, 512]      (6 SBUF tiles of [128, 512])
  gates/P       [3072, 512]     (24 tiles of [128, 512])
  h, c          [768, 512]      (6 tiles each)

For each window size w in (3,5,7):
  P = Wih @ xT + (b_ih + b_hh)  computed ONCE (shared by all w steps; step t
  just reads P shifted by (t - w//2) columns).  Step t updates only the column
  range [s, e) that is "valid" for that offset, so out-of-range window slots
  never touch state -- this reproduces the reference's mask semantics with no
  mask tensors at all.  Step 0 has h=0 so its hidden matmul is skipped.

Recurrence per step (t >= 1):  gates_psum = WhhT.T @ h  (24 [128,512] psum
tiles, 6 K-chunks each, bf16 x bf16 -> fp32 PSUM), then per d-chunk:
  pre_g = psum + P_shift (DVE)  ->  sigmoid/tanh (ACT)  ->
  c = f*c + i*g (DVE, fp32)     ->  h = o * tanh(c) (DVE, bf16)

After the 3 windows: attn logits via elementwise mul + ones-matmul column
reduction, 3-way softmax on [1,512] rows, attention weights broadcast across
partitions with a K=1 outer-product matmul, and the residual is folded into
the head matmul: logits = lin_w @ xT + lin_w @ (sum_k attn_k * locals_k) + b.

Weights are converted to bf16 on the host; matmul accumulation is fp32 in
PSUM; the cell state c stays fp32; attention/head matmuls run fp32r.
"""

import math
import numpy as np
import ml_dtypes

import concourse.bacc as bacc
import concourse.bass as bass
import concourse.tile as tile
from concourse import mybir
from concourse import bass_utils

B, L, D = 8, 512, 768
NL = 9
WINDOWS = (3, 5, 7)
NW = len(WINDOWS)
G4 = 4 * D          # 3072
P = 128
ND = D // P         # 6 d-chunks
NM = G4 // P        # 24 gate-chunks
N_CORES = 8

F32 = mybir.dt.float32
F32R = mybir.dt.float32r
BF16 = mybir.dt.bfloat16
AF = mybir.ActivationFunctionType


def _emit(tc, io):
    nc = tc.nc
    from contextlib import ExitStack

    with ExitStack() as ctx:
        const = ctx.enter_context(tc.tile_pool(name="const", bufs=1))
        wpool = ctx.enter_context(tc.tile_pool(name="wpool", bufs=1))
        ppool = ctx.enter_context(tc.tile_pool(name="ppool", bufs=1))
        state = ctx.enter_context(tc.tile_pool(name="state", bufs=1))
        post = ctx.enter_context(tc.tile_pool(name="post", bufs=8))
        tmp = ctx.enter_context(tc.tile_pool(name="tmp", bufs=6))
        attn = ctx.enter_context(tc.tile_pool(name="attn", bufs=7))
        logp = ctx.enter_context(tc.tile_pool(name="logp", bufs=1))
        psum = ctx.enter_context(tc.tile_pool(name="psum", bufs=8, space="PSUM"))

        # ---- constants / inputs resident in SBUF ----
        xf = []   # x.T fp32, for attention dot + residual head matmul
        xb = []   # x.T bf16, rhs of the input projections
        for dc in range(ND):
            t_f = const.tile([P, L], F32, tag=f"xf{dc}")
            nc.sync.dma_start(t_f, io["xf"].ap()[dc * P:(dc + 1) * P, :])
            xf.append(t_f)
            t_b = const.tile([P, L], BF16, tag=f"xb{dc}")
            nc.sync.dma_start(t_b, io["xb"].ap()[dc * P:(dc + 1) * P, :])
            xb.append(t_b)

        # combined LSTM bias, laid out [128, NW, NM]: partition p, window k,
        # gate-chunk m  <-  bias[k, m*128 + p]
        bias_sb = const.tile([P, NW, NM], F32, tag="bias")
        nc.sync.dma_start(
            bias_sb, io["bias"].ap().rearrange("k (m p) -> p k m", p=P)
        )

        lw = []
        for dc in range(ND):
            t = const.tile([P, NL], F32, tag=f"lw{dc}")
            nc.sync.dma_start(t, io["lwt"].ap()[dc * P:(dc + 1) * P, :])
            lw.append(t)
        lb_sb = const.tile([NL, 1], F32, tag="lb")
        nc.sync.dma_start(lb_sb, io["lb"].ap().rearrange("(c o) -> c o", o=1))

        ident_sb = const.tile([P, P], BF16, tag="ident")
        nc.sync.dma_start(ident_sb, io["ident"].ap())

        ones_col = const.tile([P, 1], F32, tag="ones_col")
        nc.vector.memset(ones_col, 1.0)
        ones_row = const.tile([1, P], F32, tag="ones_row")
        nc.vector.memset(ones_row, 1.0)

        locals_k = []   # per window: list of 6 bf16 [128, 512] tiles (final h)
        a_sb = []       # per-window attention logit rows [1, 512]
        inv_sqrt_d = 1.0 / math.sqrt(D)

        for k, w in enumerate(WINDOWS):
            hw_ = w // 2

            # ---- weights for this window (2 rotating 9.4MB slots) ----
            wih = []
            for kc in range(ND):
                t = wpool.tile([P, G4], BF16, tag=f"A{kc}")
                nc.sync.dma_start(t, io["wih"].ap()[k, kc * P:(kc + 1) * P, :])
                wih.append(t)
            whh = []
            for kc in range(ND):
                t = wpool.tile([P, G4], BF16, tag=f"B{kc}")
                nc.sync.dma_start(t, io["whh"].ap()[k, kc * P:(kc + 1) * P, :])
                whh.append(t)

            # ---- input projection: P_m = bias_m + sum_kc Wih[kc,m].T @ xT ----
            Pt = []
            for m in range(NM):
                ps = psum.tile([P, L], F32, tag="g")
                for kc in range(ND):
                    nc.tensor.matmul(
                        ps,
                        lhsT=wih[kc][:, m * P:(m + 1) * P],
                        rhs=xb[kc][:],
                        start=(kc == 0),
                        stop=(kc == ND - 1),
                    )
                pt = ppool.tile([P, L], BF16, tag=f"P{m}")
                nc.scalar.activation(
                    pt, ps, AF.Identity, bias=bias_sb[:, k, m:m + 1], scale=1.0
                )
                Pt.append(pt)

            # ---- state init ----
            c = []
            h = []
            for dc in range(ND):
                ct = state.tile([P, L], F32, tag=f"c{dc}")
                nc.gpsimd.memset(ct, 0.0)
                c.append(ct)
                ht = state.tile([P, L], BF16, tag=f"loc{k}_{dc}")
                nc.gpsimd.memset(ht, 0.0)
                h.append(ht)

            # ---- recurrence over window positions ----
            for t in range(w):
                off = t - hw_
                s = max(0, -off)
                e = min(L, L - off)
                n = e - s

                if t == 0:
                    # h == 0: gates come straight from P (bias included)
                    for dc in range(ND):
                        i_t = post.tile([P, L], BF16, tag="post")
                        nc.scalar.activation(
                            i_t[:, :n], Pt[0 + dc][:, s + off:e + off], AF.Sigmoid
                        )
                        g_t = post.tile([P, L], BF16, tag="post")
                        nc.scalar.activation(
                            g_t[:, :n], Pt[12 + dc][:, s + off:e + off], AF.Tanh
                        )
                        o_t = post.tile([P, L], BF16, tag="post")
                        nc.scalar.activation(
                            o_t[:, :n], Pt[18 + dc][:, s + off:e + off], AF.Sigmoid
                        )
                        nc.vector.tensor_mul(c[dc][:, s:e], i_t[:, :n], g_t[:, :n])
                        tc_t = post.tile([P, L], BF16, tag="post")
                        nc.scalar.activation(tc_t[:, :n], c[dc][:, s:e], AF.Tanh)
                        nc.vector.tensor_mul(h[dc][:, s:e], o_t[:, :n], tc_t[:, :n])
                    continue

                for dc in range(ND):
                    # 4 gate psum tiles for this d-chunk: i, f, g, o.
                    # P_shift (incl. bias) is folded into the accumulation
                    # with an identity matmul, so ACT reads gates from PSUM.
                    gp = []
                    for base in (0, 6, 12, 18):
                        m = base + dc
                        ps = psum.tile([P, L], F32, tag="g")
                        nc.tensor.matmul(
                            ps[:, s:e],
                            lhsT=ident_sb[:],
                            rhs=Pt[m][:, s + off:e + off],
                            start=True,
                            stop=False,
                        )
                        for kc in range(ND):
                            nc.tensor.matmul(
                                ps[:, s:e],
                                lhsT=whh[kc][:, m * P:(m + 1) * P],
                                rhs=h[kc][:, s:e],
                                start=False,
                                stop=(kc == ND - 1),
                            )
                        gp.append(ps)

                    acts = []
                    for gi, fn in enumerate(
                        (AF.Sigmoid, AF.Sigmoid, AF.Tanh, AF.Sigmoid)
                    ):
                        a = post.tile([P, L], BF16, tag="post")
                        nc.scalar.activation(a[:, :n], gp[gi][:, s:e], fn)
                        acts.append(a)
                    i_t, f_t, g_t, o_t = acts

                    t1 = tmp.tile([P, L], F32, tag="tmp")
                    nc.vector.tensor_mul(t1[:, :n], i_t[:, :n], g_t[:, :n])
                    t2 = tmp.tile([P, L], F32, tag="tmp")
                    nc.vector.tensor_mul(t2[:, :n], f_t[:, :n], c[dc][:, s:e])
                    nc.vector.tensor_add(c[dc][:, s:e], t1[:, :n], t2[:, :n])
                    tc_t = post.tile([P, L], BF16, tag="post")
                    nc.scalar.activation(tc_t[:, :n], c[dc][:, s:e], AF.Tanh)
                    nc.vector.tensor_mul(h[dc][:, s:e], o_t[:, :n], tc_t[:, :n])

            locals_k.append(h)

            # attention dot for this window, overlapped with the next window
            psd = psum.tile([1, L], F32, tag="g")
            for dc in range(ND):
                td = tmp.tile([P, L], F32, tag="tmp")
                nc.vector.tensor_mul(td, xf[dc][:], h[dc][:])
                nc.tensor.matmul(
                    psd,
                    lhsT=ones_col[:],
                    rhs=td[:],
                    start=(dc == 0),
                    stop=(dc == ND - 1),
                )
            ak = attn.tile([1, L], F32, tag=f"ak{k}", bufs=1)
            nc.scalar.activation(ak, psd, AF.Copy, scale=inv_sqrt_d)
            a_sb.append(ak)

        # ---- attention over the 3 window outputs ----
        mx1 = attn.tile([1, L], F32, tag="sm")
        nc.vector.tensor_max(mx1, a_sb[0][:], a_sb[1][:])
        mx2 = attn.tile([1, L], F32, tag="sm")
        nc.vector.tensor_max(mx2, mx1[:], a_sb[2][:])
        d_sb = []
        for k in range(NW):
            d_k = attn.tile([1, L], F32, tag="sm")
            nc.vector.tensor_sub(d_k, a_sb[k][:], mx2[:])
            d_sb.append(d_k)
        e_sb = []
        for k in range(NW):
            ek = attn.tile([1, L], F32, tag="sm")
            nc.scalar.activation(ek, d_sb[k][:], AF.Exp)
            e_sb.append(ek)
        s1 = attn.tile([1, L], F32, tag="sm")
        nc.vector.tensor_add(s1, e_sb[0][:], e_sb[1][:])
        s2 = attn.tile([1, L], F32, tag="sm")
        nc.vector.tensor_add(s2, s1[:], e_sb[2][:])
        r = attn.tile([1, L], F32, tag="sm")
        nc.vector.reciprocal(r, s2[:])

        wb = []   # attention weights broadcast to [128, 512] (PSUM)
        for k in range(NW):
            wk = attn.tile([1, L], F32, tag="sm")
            nc.vector.tensor_mul(wk, e_sb[k][:], r[:])
            pb = psum.tile([P, L], F32, tag="g")
            nc.tensor.matmul(
                pb,
                lhsT=ones_row[:],
                rhs=wk[:],
                start=True,
                stop=True,
            )
            wb.append(pb)

        # ---- head: logits = lin_w @ (x + sum_k attn_k * locals_k) + b ----
        ps_log = psum.tile([NL, L], F32, tag="g")
        for dc in range(ND):
            nc.tensor.matmul(
                ps_log,
                lhsT=lw[dc][:],
                rhs=xf[dc][:],
                start=(dc == 0),
                stop=False,
            )
        for dc in range(ND):
            lf = tmp.tile([P, L], F32, tag="tmp")
            nc.vector.tensor_mul(lf, wb[0][:], locals_k[0][dc][:])
            t3 = tmp.tile([P, L], F32, tag="tmp")
            nc.vector.tensor_mul(t3, wb[1][:], locals_k[1][dc][:])
            lf2 = tmp.tile([P, L], F32, tag="tmp")
            nc.vector.tensor_add(lf2, lf[:], t3[:])
            t4 = tmp.tile([P, L], F32, tag="tmp")
            nc.vector.tensor_mul(t4, wb[2][:], locals_k[2][dc][:])
            lf3 = tmp.tile([P, L], F32, tag="tmp")
            nc.vector.tensor_add(lf3, lf2[:], t4[:])
            nc.tensor.matmul(
                ps_log,
                lhsT=lw[dc][:],
                rhs=lf3[:],
                start=False,
                stop=(dc == ND - 1),
            )
        logits = logp.tile([NL, L], F32, tag="logits")
        nc.scalar.activation(logits, ps_log, AF.Identity, bias=lb_sb[:, 0:1])
        # store transposed: out[l, c] = logits[c, l]
        nc.sync.dma_start(io["out"].ap().rearrange("l c -> c l"), logits[:])


_NC_CACHE = {}


def _get_nc():
    if "nc" not in _NC_CACHE:
        nc = bacc.Bacc("TRN2", target_bir_lowering=False, debug=False)
        io = {
            "xf": nc.dram_tensor("xf", [D, L], F32, kind="ExternalInput"),
            "xb": nc.dram_tensor("xb", [D, L], BF16, kind="ExternalInput"),
            "wih": nc.dram_tensor("wih", [NW, D, G4], BF16, kind="ExternalInput"),
            "whh": nc.dram_tensor("whh", [NW, D, G4], BF16, kind="ExternalInput"),
            "bias": nc.dram_tensor("bias", [NW, G4], F32, kind="ExternalInput"),
            "lwt": nc.dram_tensor("lwt", [D, NL], F32, kind="ExternalInput"),
            "lb": nc.dram_tensor("lb", [NL], F32, kind="ExternalInput"),
            "ident": nc.dram_tensor("ident", [P, P], BF16, kind="ExternalInput"),
            "out": nc.dram_tensor("out", [L, NL], F32, kind="ExternalOutput"),
        }
        with tile.TileContext(nc) as tc:
            _emit(tc, io)
        nc.compile()
        _NC_CACHE["nc"] = nc
    return _NC_CACHE["nc"]


def _in_maps(sequence_output, W_ih, W_hh, b_ih, b_hh, lin_w, lin_b):
    x = np.asarray(sequence_output, np.float32)
    WihT = np.ascontiguousarray(
        np.transpose(np.asarray(W_ih, np.float32), (0, 2, 1))
    ).astype(ml_dtypes.bfloat16)
    WhhT = np.ascontiguousarray(
        np.transpose(np.asarray(W_hh, np.float32), (0, 2, 1))
    ).astype(ml_dtypes.bfloat16)
    biasc = np.asarray(b_ih, np.float32) + np.asarray(b_hh, np.float32)
    lwt = np.ascontiguousarray(np.asarray(lin_w, np.float32).T)
    lb = np.asarray(lin_b, np.float32)
    maps = []
    for b in range(B):
        xT = np.ascontiguousarray(x[b].T)
        maps.append({
            "xf": xT,
            "xb": xT.astype(ml_dtypes.bfloat16),
            "wih": WihT,
            "whh": WhhT,
            "bias": biasc,
            "lwt": lwt,
            "lb": lb,
            "ident": np.eye(P, dtype=np.float32).astype(ml_dtypes.bfloat16),
        })
    return maps


def kernel(sequence_output, W_ih, W_hh, b_ih, b_hh, lin_w, lin_b):
    nc = _get_nc()
    maps = _in_maps(sequence_output, W_ih, W_hh, b_ih, b_hh, lin_w, lin_b)
    res = bass_utils.run_bass_kernel_spmd(nc, maps, core_ids=list(range(N_CORES)))
    return np.stack([res.results[b]["out"] for b in range(B)], axis=0)


def run_traced(inputs, **kw):
    """For test.py: run with NTFF tracing, returns BassKernelResults."""
    nc = _get_nc()
    maps = _in_maps(**inputs)
    return bass_utils.run_bass_kernel_spmd(
        nc, maps, core_ids=list(range(N_CORES)), trace=True, **kw
    )


# revision 30
# speedup vs baseline: 1.0665x; 1.0665x over previous
"""Trainium2 Bass kernel: windowed-LSTM local attention + linear head (LBNER).

Strategy
--------
Data-parallel over batch: B=8 sequences -> 8 NeuronCores, one sequence each.
Per core everything is laid out feature-on-partitions, L=512 on the free dim:

  xT            [768, 512]      (6 SBUF tiles of [128, 512])
  gates/P       [3072, 512]     (24 tiles of [128, 512])
  h, c          [768, 512]      (6 tiles each)

For each window size w in (3,5,7):
  P = Wih @ xT + (b_ih + b_hh)  computed ONCE (shared by all w steps; step t
  just reads P shifted by (t - w//2) columns).  Step t updates only the column
  range [s, e) that is "valid" for that offset, so out-of-range window slots
  never touch state -- this reproduces the reference's mask semantics with no
  mask tensors at all.  Step 0 has h=0 so its hidden matmul is skipped.

Recurrence per step (t >= 1):  gates_psum = WhhT.T @ h  (24 [128,512] psum
tiles, 6 K-chunks each, bf16 x bf16 -> fp32 PSUM), then per d-chunk:
  pre_g = psum + P_shift (DVE)  ->  sigmoid/tanh (ACT)  ->
  c = f*c + i*g (DVE, fp32)     ->  h = o * tanh(c) (DVE, bf16)

After the 3 windows: attn logits via elementwise mul + ones-matmul column
reduction, 3-way softmax on [1,512] rows, attention weights broadcast across
partitions with a K=1 outer-product matmul, and the residual is folded into
the head matmul: logits = lin_w @ xT + lin_w @ (sum_k attn_k * locals_k) + b.

Weights are converted to bf16 on the host; matmul accumulation is fp32 in
PSUM; the cell state c stays fp32; attention/head matmuls run plain fp32.
"""

import math
import numpy as np
import ml_dtypes

import concourse.bacc as bacc
import concourse.bass as bass
import concourse.tile as tile
from concourse import mybir
from concourse import bass_utils

B, L, D = 8, 512, 768
NL = 9
WINDOWS = (3, 5, 7)
NW = len(WINDOWS)
G4 = 4 * D          # 3072
P = 128
ND = D // P         # 6 d-chunks
NM = G4 // P        # 24 gate-chunks
N_CORES = 8

F32 = mybir.dt.float32
F32R = mybir.dt.float32r
BF16 = mybir.dt.bfloat16
AF = mybir.ActivationFunctionType


def _emit(tc, io):
    nc = tc.nc
    from contextlib import ExitStack

    with ExitStack() as ctx:
        const = ctx.enter_context(tc.tile_pool(name="const", bufs=1))
        wpool = ctx.enter_context(tc.tile_pool(name="wpool", bufs=1))
        ppool = ctx.enter_context(tc.tile_pool(name="ppool", bufs=1))
        state = ctx.enter_context(tc.tile_pool(name="state", bufs=1))
        post = ctx.enter_context(tc.tile_pool(name="post", bufs=8))
        tmp = ctx.enter_context(tc.tile_pool(name="tmp", bufs=6))
        attn = ctx.enter_context(tc.tile_pool(name="attn", bufs=7))
        logp = ctx.enter_context(tc.tile_pool(name="logp", bufs=1))
        psum = ctx.enter_context(tc.tile_pool(name="psum", bufs=8, space="PSUM"))

        # ---- constants / inputs resident in SBUF ----
        xf = []   # x.T fp32, for attention dot + residual head matmul
        xb = []   # x.T bf16, rhs of the input projections
        for dc in range(ND):
            t_f = const.tile([P, L], F32, tag=f"xf{dc}")
            nc.sync.dma_start(t_f, io["xf"].ap()[dc * P:(dc + 1) * P, :])
            xf.append(t_f)
            t_b = const.tile([P, L], BF16, tag=f"xb{dc}")
            nc.sync.dma_start(t_b, io["xb"].ap()[dc * P:(dc + 1) * P, :])
            xb.append(t_b)

        # combined LSTM bias, laid out [128, NW, NM]: partition p, window k,
        # gate-chunk m  <-  bias[k, m*128 + p]
        bias_sb = const.tile([P, NW, NM], F32, tag="bias")
        nc.sync.dma_start(
            bias_sb, io["bias"].ap().rearrange("k (m p) -> p k m", p=P)
        )

        lw = []
        for dc in range(ND):
            t = const.tile([P, NL], F32, tag=f"lw{dc}")
            nc.sync.dma_start(t, io["lwt"].ap()[dc * P:(dc + 1) * P, :])
            lw.append(t)
        lb_sb = const.tile([NL, 1], F32, tag="lb")
        nc.sync.dma_start(lb_sb, io["lb"].ap().rearrange("(c o) -> c o", o=1))

        ident_sb = const.tile([P, P], BF16, tag="ident")
        nc.sync.dma_start(ident_sb, io["ident"].ap())

        ones_col = const.tile([P, 1], F32, tag="ones_col")
        nc.vector.memset(ones_col, 1.0)
        ones_row = const.tile([1, P], F32, tag="ones_row")
        nc.vector.memset(ones_row, 1.0)

        locals_k = []   # per window: list of 6 bf16 [128, 512] tiles (final h)
        a_sb = []       # per-window attention logit rows [1, 512]
        inv_sqrt_d = 1.0 / math.sqrt(D)

        for k, w in enumerate(WINDOWS):
            hw_ = w // 2

            # ---- weights for this window (2 rotating 9.4MB slots) ----
            wih = []
            for kc in range(ND):
                t = wpool.tile([P, G4], BF16, tag=f"A{kc}")
                nc.sync.dma_start(t, io["wih"].ap()[k, kc * P:(kc + 1) * P, :])
                wih.append(t)
            whh = []
            for kc in range(ND):
                t = wpool.tile([P, G4], BF16, tag=f"B{kc}")
                nc.sync.dma_start(t, io["whh"].ap()[k, kc * P:(kc + 1) * P, :])
                whh.append(t)

            # ---- input projection: P_m = bias_m + sum_kc Wih[kc,m].T @ xT ----
            Pt = []
            for m in range(NM):
                ps = psum.tile([P, L], F32, tag="g")
                for kc in range(ND):
                    nc.tensor.matmul(
                        ps,
                        lhsT=wih[kc][:, m * P:(m + 1) * P],
                        rhs=xb[kc][:],
                        start=(kc == 0),
                        stop=(kc == ND - 1),
                    )
                pt = ppool.tile([P, L], BF16, tag=f"P{m}")
                nc.scalar.activation(
                    pt, ps, AF.Identity, bias=bias_sb[:, k, m:m + 1], scale=1.0
                )
                Pt.append(pt)

            # ---- state init ----
            c = []
            h = []
            for dc in range(ND):
                ct = state.tile([P, L], F32, tag=f"c{dc}")
                nc.gpsimd.memset(ct, 0.0)
                c.append(ct)
                ht = state.tile([P, L], BF16, tag=f"loc{k}_{dc}")
                nc.gpsimd.memset(ht, 0.0)
                h.append(ht)

            # ---- recurrence over window positions ----
            for t in range(w):
                off = t - hw_
                s = max(0, -off)
                e = min(L, L - off)
                n = e - s

                if t == 0:
                    # h == 0: gates come straight from P (bias included)
                    for dc in range(ND):
                        i_t = post.tile([P, L], BF16, tag="post")
                        nc.scalar.activation(
                            i_t[:, :n], Pt[0 + dc][:, s + off:e + off], AF.Sigmoid
                        )
                        g_t = post.tile([P, L], BF16, tag="post")
                        nc.scalar.activation(
                            g_t[:, :n], Pt[12 + dc][:, s + off:e + off], AF.Tanh
                        )
                        o_t = post.tile([P, L], BF16, tag="post")
                        nc.scalar.activation(
                            o_t[:, :n], Pt[18 + dc][:, s + off:e + off], AF.Sigmoid
                        )
                        nc.vector.tensor_mul(c[dc][:, s:e], i_t[:, :n], g_t[:, :n])
                        tc_t = post.tile([P, L], BF16, tag="post")
                        nc.scalar.activation(tc_t[:, :n], c[dc][:, s:e], AF.Tanh)
                        nc.vector.tensor_mul(h[dc][:, s:e], o_t[:, :n], tc_t[:, :n])
                    continue

                for dc in range(ND):
                    # 4 gate psum tiles for this d-chunk: i, f, g, o.
                    # P_shift (incl. bias) is folded into the accumulation
                    # with an identity matmul, so ACT reads gates from PSUM.
                    gp = []
                    for base in (0, 6, 12, 18):
                        m = base + dc
                        ps = psum.tile([P, L], F32, tag="g")
                        nc.tensor.matmul(
                            ps[:, s:e],
                            lhsT=ident_sb[:],
                            rhs=Pt[m][:, s + off:e + off],
                            start=True,
                            stop=False,
                        )
                        for kc in range(ND):
                            nc.tensor.matmul(
                                ps[:, s:e],
                                lhsT=whh[kc][:, m * P:(m + 1) * P],
                                rhs=h[kc][:, s:e],
                                start=False,
                                stop=(kc == ND - 1),
                            )
                        gp.append(ps)

                    acts = []
                    for gi, fn in enumerate(
                        (AF.Sigmoid, AF.Sigmoid, AF.Tanh, AF.Sigmoid)
                    ):
                        a = post.tile([P, L], BF16, tag="post")
                        nc.scalar.activation(a[:, :n], gp[gi][:, s:e], fn)
                        acts.append(a)
                    i_t, f_t, g_t, o_t = acts

                    t1 = tmp.tile([P, L], F32, tag="tmp")
                    nc.vector.tensor_mul(t1[:, :n], i_t[:, :n], g_t[:, :n])
                    t2 = tmp.tile([P, L], F32, tag="tmp")
                    nc.vector.tensor_mul(t2[:, :n], f_t[:, :n], c[dc][:, s:e])
                    nc.vector.tensor_add(c[dc][:, s:e], t1[:, :n], t2[:, :n])
                    tc_t = post.tile([P, L], BF16, tag="post")
                    nc.scalar.activation(tc_t[:, :n], c[dc][:, s:e], AF.Tanh)
                    nc.vector.tensor_mul(h[dc][:, s:e], o_t[:, :n], tc_t[:, :n])

            locals_k.append(h)

            # attention dot for this window, overlapped with the next window
            psd = psum.tile([1, L], F32, tag="g")
            for dc in range(ND):
                td = tmp.tile([P, L], F32, tag="tmp")
                nc.vector.tensor_mul(td, xf[dc][:], h[dc][:])
                nc.tensor.matmul(
                    psd,
                    lhsT=ones_col[:],
                    rhs=td[:],
                    start=(dc == 0),
                    stop=(dc == ND - 1),
                )
            ak = attn.tile([1, L], F32, tag=f"ak{k}", bufs=1)
            nc.scalar.activation(ak, psd, AF.Copy, scale=inv_sqrt_d)
            a_sb.append(ak)

        # ---- attention over the 3 window outputs ----
        mx1 = attn.tile([1, L], F32, tag="sm")
        nc.vector.tensor_max(mx1, a_sb[0][:], a_sb[1][:])
        mx2 = attn.tile([1, L], F32, tag="sm")
        nc.vector.tensor_max(mx2, mx1[:], a_sb[2][:])
        d_sb = []
        for k in range(NW):
            d_k = attn.tile([1, L], F32, tag="sm")
            nc.vector.tensor_sub(d_k, a_sb[k][:], mx2[:])
            d_sb.append(d_k)
        e_sb = []
        for k in range(NW):
            ek = attn.tile([1, L], F32, tag="sm")
            nc.scalar.activation(ek, d_sb[k][:], AF.Exp)
            e_sb.append(ek)
        s1 = attn.tile([1, L], F32, tag="sm")
        nc.vector.tensor_add(s1, e_sb[0][:], e_sb[1][:])
        s2 = attn.tile([1, L], F32, tag="sm")
        nc.vector.tensor_add(s2, s1[:], e_sb[2][:])
        r = attn.tile([1, L], F32, tag="sm")
        nc.vector.reciprocal(r, s2[:])

        wb = []   # attention weights broadcast to [128, 512] (PSUM)
        for k in range(NW):
            wk = attn.tile([1, L], F32, tag="sm")
            nc.vector.tensor_mul(wk, e_sb[k][:], r[:])
            pb = psum.tile([P, L], F32, tag="g")
            nc.tensor.matmul(
                pb,
                lhsT=ones_row[:],
                rhs=wk[:],
                start=True,
                stop=True,
            )
            wb.append(pb)

        # ---- head: logits = lin_w @ (x + sum_k attn_k * locals_k) + b ----
        ps_log = psum.tile([NL, L], F32, tag="g")
        for dc in range(ND):
            nc.tensor.matmul(
                ps_log,
                lhsT=lw[dc][:],
                rhs=xf[dc][:],
                start=(dc == 0),
                stop=False,
            )
        for dc in range(ND):
            lf = tmp.tile([P, L], F32, tag="tmp")
            nc.vector.tensor_mul(lf, wb[0][:], locals_k[0][dc][:])
            t3 = tmp.tile([P, L], F32, tag="tmp")
            nc.vector.tensor_mul(t3, wb[1][:], locals_k[1][dc][:])
            lf2 = tmp.tile([P, L], F32, tag="tmp")
            nc.vector.tensor_add(lf2, lf[:], t3[:])
            t4 = tmp.tile([P, L], F32, tag="tmp")
            nc.vector.tensor_mul(t4, wb[2][:], locals_k[2][dc][:])
            lf3 = tmp.tile([P, L], F32, tag="tmp")
            nc.vector.tensor_add(lf3, lf2[:], t4[:])
            nc.tensor.matmul(
                ps_log,
                lhsT=lw[dc][:],
                rhs=lf3[:],
                start=False,
                stop=(dc == ND - 1),
            )
        logits = logp.tile([NL, L], F32, tag="logits")
        nc.scalar.activation(logits, ps_log, AF.Identity, bias=lb_sb[:, 0:1])
        # store transposed: out[l, c] = logits[c, l]
        nc.sync.dma_start(io["out"].ap().rearrange("l c -> c l"), logits[:])


_NC_CACHE = {}


def _get_nc():
    if "nc" not in _NC_CACHE:
        nc = bacc.Bacc("TRN2", target_bir_lowering=False, debug=False)
        io = {
            "xf": nc.dram_tensor("xf", [D, L], F32, kind="ExternalInput"),
            "xb": nc.dram_tensor("xb", [D, L], BF16, kind="ExternalInput"),
            "wih": nc.dram_tensor("wih", [NW, D, G4], BF16, kind="ExternalInput"),
            "whh": nc.dram_tensor("whh", [NW, D, G4], BF16, kind="ExternalInput"),
            "bias": nc.dram_tensor("bias", [NW, G4], F32, kind="ExternalInput"),
            "lwt": nc.dram_tensor("lwt", [D, NL], F32, kind="ExternalInput"),
            "lb": nc.dram_tensor("lb", [NL], F32, kind="ExternalInput"),
            "ident": nc.dram_tensor("ident", [P, P], BF16, kind="ExternalInput"),
            "out": nc.dram_tensor("out", [L, NL], F32, kind="ExternalOutput"),
        }
        with tile.TileContext(nc) as tc:
            _emit(tc, io)
        nc.compile()
        _NC_CACHE["nc"] = nc
    return _NC_CACHE["nc"]


def _in_maps(sequence_output, W_ih, W_hh, b_ih, b_hh, lin_w, lin_b):
    x = np.asarray(sequence_output, np.float32)
    WihT = np.ascontiguousarray(
        np.transpose(np.asarray(W_ih, np.float32), (0, 2, 1))
    ).astype(ml_dtypes.bfloat16)
    WhhT = np.ascontiguousarray(
        np.transpose(np.asarray(W_hh, np.float32), (0, 2, 1))
    ).astype(ml_dtypes.bfloat16)
    biasc = np.asarray(b_ih, np.float32) + np.asarray(b_hh, np.float32)
    lwt = np.ascontiguousarray(np.asarray(lin_w, np.float32).T)
    lb = np.asarray(lin_b, np.float32)
    maps = []
    for b in range(B):
        xT = np.ascontiguousarray(x[b].T)
        maps.append({
            "xf": xT,
            "xb": xT.astype(ml_dtypes.bfloat16),
            "wih": WihT,
            "whh": WhhT,
            "bias": biasc,
            "lwt": lwt,
            "lb": lb,
            "ident": np.eye(P, dtype=np.float32).astype(ml_dtypes.bfloat16),
        })
    return maps


def kernel(sequence_output, W_ih, W_hh, b_ih, b_hh, lin_w, lin_b):
    nc = _get_nc()
    maps = _in_maps(sequence_output, W_ih, W_hh, b_ih, b_hh, lin_w, lin_b)
    res = bass_utils.run_bass_kernel_spmd(nc, maps, core_ids=list(range(N_CORES)))
    return np.stack([res.results[b]["out"] for b in range(B)], axis=0)


def run_traced(inputs, **kw):
    """For test.py: run with NTFF tracing, returns BassKernelResults."""
    nc = _get_nc()
    maps = _in_maps(**inputs)
    return bass_utils.run_bass_kernel_spmd(
        nc, maps, core_ids=list(range(N_CORES)), trace=True, **kw
    )
